# revision 42
# baseline (speedup 1.0000x reference)
"""FFTConv1d-with-threshold kernel for Trainium2, 8 NeuronCores.

Math: the reference (flat 16900-pt FFT -> prune coeffs with |Re|<0.01 ->
multiply by kernel FFT -> iFFT -> roll -> channel-sum -> slice) is
algebraically a standard 3x3 pad-1 conv2d applied to (xp - delta), where
delta is the inverse FFT of the pruned (below-threshold) coefficients.

Device algorithm per core (core = (batch b, out-channel half)):
  16900-pt FFT via Cooley-Tukey (100 x 169) as dense PE matmuls, prune
  mask on DVE, inverse transform (fp32r), subtract, then the 3x3 conv as
  9 PSUM-accumulated matmuls (fp32r) with per-tap shifted APs.
Forward transform runs in fp32 (threshold decisions need ~1e-4 absolute
accuracy); everything downstream of the mask only shapes delta (~0.6% of
the signal), so fp32r suffices there.
"""

import numpy as np

import bass_rust
import concourse.bass as bass
import concourse.mybir as mybir
from concourse.bass_utils import run_bass_kernel_spmd
from concourse.masks import make_identity
from concourse.tile import TileContext

F32 = mybir.dt.float32
F32R = mybir.dt.float32r
BF16 = mybir.dt.bfloat16

N1, N2, N = 100, 169, 16900
W130 = 130
B, C, O = 4, 32, 32
OH = O // 2
THRESH = 0.01
S = C
FS1 = 338            # stage-1 F chunk: 2 channels x 169
NF1 = (S * N2) // FS1
FS2 = 400            # stage-2 F chunk: 4 channels x 100
NF2 = (S * N1) // FS2
QCH = [(0, 128), (128, 41)]


def _split_excess_waits(nc):
    # This walrus build accepts 1 sync-wait slot per instruction; Tile can
    # attach several. Move extras onto nofuse NOPs on the same engine.
    for f in nc.m.functions:
        for blk in f.blocks:
            insts = blk.instructions
            changed = False
            new_list = []
            for inst in insts:
                si = inst.sync_info
                if si is not None and len(si.on_wait) > 1:
                    waits = list(si.on_wait)
                    extra, keep = waits[:-1], waits[-1:]
                    for k, w in enumerate(extra):
                        new_list.append(bass_rust.InstNoOp(
                            name=f"{inst.name}-ws{k}",
                            engine=inst.engine,
                            ins=[], outs=[], bass_nofuse=True,
                            sync_info=bass_rust.SyncInfo(on_wait=[w], on_update=[]),
                        ))
                    inst.sync_info = bass_rust.SyncInfo(
                        on_wait=keep, on_update=list(si.on_update))
                    changed = True
                new_list.append(inst)
            if changed:
                blk.instructions = new_list


def _build():
    nc = bass.Bass("TRN2")
    x0 = nc.dram_tensor("x0", [N1, S * N2], F32, kind="ExternalInput")
    wk = nc.dram_tensor("wk", [96, 3 * OH], F32, kind="ExternalInput")
    bias_h = nc.dram_tensor("bias_h", [OH, 1], F32, kind="ExternalInput")
    cn = {}
    for name, shape in [
        ("c100c", [N1, N1]), ("c100sn", [N1, N1]),
        ("c169c", [N2, N2]), ("c169s", [N2, N2]), ("c169sn", [N2, N2]),
        ("t1c", [N1, N2]), ("t1s", [N1, N2]), ("t1sn", [N1, N2]),
        ("t2c", [N2, N1]), ("t2s", [N2, N1]), ("t2sn", [N2, N1]),
        ("cinvc", [N1, N1]), ("cinvsn", [N1, N1]),
        ("wgtx", [85, 400]),
    ]:
        cn[name] = nc.dram_tensor(name, shape, F32, kind="ExternalInput")
    out_part = nc.dram_tensor("out_part", [OH, 128 * 128], F32, kind="ExternalOutput")
    xdram = nc.dram_tensor("xdram", [C, N], F32R)

    with TileContext(nc) as tc:
        with tc.tile_pool(name="const", bufs=1) as cst, \
             tc.tile_pool(name="big", bufs=1) as big, \
             tc.tile_pool(name="chunk", bufs=3) as chk:

            ct = {}
            for name, dram in cn.items():
                dt = BF16 if name.startswith("cinv") else F32
                dma = nc.sync if dt == F32 else nc.gpsimd
                rows = dram.shape[0]
                if rows > 128:
                    ta = cst.tile([128, dram.shape[1]], dt, tag=name + "a", name=name + "a")
                    tb = cst.tile([rows - 128, dram.shape[1]], dt, tag=name + "b", name=name + "b")
                    dma.dma_start(out=ta[:], in_=dram[0:128, :])
                    dma.dma_start(out=tb[:], in_=dram[128:rows, :])
                    ct[name] = {0: ta, 128: tb}
                else:
                    ct[name] = cst.tile(list(dram.shape), dt, tag=name, name=name)
                    dma.dma_start(out=ct[name][:], in_=dram[:])
            # bf16 copies of the 169-DFT matrices for the Im-path matmuls
            for name in ("c169c", "c169sn"):
                ta = cst.tile([128, N2], BF16, tag=name + "ha", name=name + "ha")
                tb = cst.tile([41, N2], BF16, tag=name + "hb", name=name + "hb")
                nc.gpsimd.dma_start(out=ta[:], in_=cn[name][0:128, :])
                nc.gpsimd.dma_start(out=tb[:], in_=cn[name][128:N2, :])
                ct[name + "h"] = {0: ta, 128: tb}
            # fp32r copies of the 169-DFT matrices for the inverse matmuls
            for name in ("c169c", "c169s", "c169sn"):
                ta = cst.tile([128, N2], F32R, tag=name + "ra", name=name + "ra")
                tb = cst.tile([41, N2], F32R, tag=name + "rb", name=name + "rb")
                nc.gpsimd.dma_start(out=ta[:], in_=cn[name][0:128, :])
                nc.gpsimd.dma_start(out=tb[:], in_=cn[name][128:N2, :])
                ct[name + "r"] = {0: ta, 128: tb}
            wk_t = cst.tile([96, 3 * OH], F32R, tag="wk")
            nc.gpsimd.dma_start(out=wk_t[:], in_=wk[:])
            bias_t = cst.tile([OH, 1], F32, tag="bias")
            nc.sync.dma_start(out=bias_t[:], in_=bias_h[:])
            ident = cst.tile([128, 128], F32, tag="ident")
            make_identity(nc, ident[:])
            identh = cst.tile([128, 128], BF16, tag="identh")
            make_identity(nc, identh[:])

            x0t = big.tile([N1, S * N2], F32, tag="x0")
            for f in range(NF1):
                sl = bass.ts(f, FS1)
                nc.sync.dma_start(out=x0t[:, sl], in_=x0[:, sl])

            htwtp_cm = tc.tile_pool(name="htwtp", bufs=1)
            htwtp = htwtp_cm.__enter__()
            htwt_re = htwtp.tile([N1, S * N2], BF16, tag="htwt_re")
            htwt_im = htwtp.tile([N1, S * N2], BF16, tag="htwt_im")
            gttp_cm = tc.tile_pool(name="gttp", bufs=1)
            gttp = gttp_cm.__enter__()
            gtt_re_a = gttp.tile([128, S * N1], F32, tag="gtt_re_a")
            gtt_re_b = gttp.tile([41, S * N1], F32, tag="gtt_re_b")
            gtt_im_a = gttp.tile([128, S * N1], F32, tag="gtt_im_a")
            gtt_im_b = gttp.tile([41, S * N1], F32, tag="gtt_im_b")
            gtt_reh_a = gttp.tile([128, S * N1], BF16, tag="gtt_reh_a")
            gtt_reh_b = gttp.tile([41, S * N1], BF16, tag="gtt_reh_b")
            gtt_imh_a = gttp.tile([128, S * N1], BF16, tag="gtt_imh_a")
            gtt_imh_b = gttp.tile([41, S * N1], BF16, tag="gtt_imh_b")
            # conv quarters: partition 32s+c holds channel c's flat image
            # rows [a0[g], a1[g]) shifted left by s (169-aligned base)
            QA = []
            for g in range(4):
                n_lo, n_hi = 4160 * g, 4160 * g + 4420
                a0, a1 = n_lo // N2, -(-n_hi // N2)
                QA.append((a0, a1))
            QSPAN = max((a1 - a0) * N2 for a0, a1 in QA)

            # ---------- FWD stage 1 + twiddle1 + T1 ----------
            with tc.tile_pool(name="ps1", bufs=2, space="PSUM") as ps1, \
                 tc.tile_pool(name="pt1", bufs=2, space="PSUM") as pt1:
                for f in range(NF1):
                    sl = bass.ts(f, FS1)
                    ps_re = ps1.tile([N1, FS1], F32, tag="s1re")
                    ps_im = ps1.tile([N1, FS1], F32, tag="s1im")
                    nc.tensor.matmul(ps_re[:], ct["c100c"][:], x0t[:, sl], start=True, stop=True)
                    nc.tensor.matmul(ps_im[:], ct["c100sn"][:], x0t[:, sl], start=True, stop=True)
                    g_re = chk.tile([N1, FS1], F32, tag="gt_re")
                    g_im = chk.tile([N1, FS1], F32, tag="gt_im")
                    for j in range(2):
                        cs = bass.ts(j, N2)
                        tmp1 = chk.tile([N1, N2], F32, tag="tw1tmp")
                        tmp2 = chk.tile([N1, N2], F32, tag="tw1tmp2")
                        tmp3 = chk.tile([N1, N2], F32, tag="tw1tmp3")
                        tmp4 = chk.tile([N1, N2], F32, tag="tw1tmp4")
                        # Gt_re = Gre*t1c + Gim*t1s ; Gt_im = Gim*t1c + Gre*(-t1s)
                        nc.vector.tensor_mul(out=tmp1[:], in0=ps_re[:, cs], in1=ct["t1c"][:])
                        nc.vector.tensor_mul(out=tmp2[:], in0=ps_im[:, cs], in1=ct["t1s"][:])
                        nc.gpsimd.tensor_add(out=g_re[:, cs], in0=tmp1[:], in1=tmp2[:])
                        nc.vector.tensor_mul(out=tmp3[:], in0=ps_im[:, cs], in1=ct["t1c"][:])
                        nc.vector.tensor_mul(out=tmp4[:], in0=ps_re[:, cs], in1=ct["t1sn"][:])
                        nc.gpsimd.tensor_add(out=g_im[:, cs], in0=tmp3[:], in1=tmp4[:])
                    for j in range(2):
                        c = 2 * f + j
                        for g_src, dst_a, dst_b in ((g_re, gtt_re_a, gtt_re_b),
                                                    (g_im, gtt_im_a, gtt_im_b)):
                            pta = pt1.tile([128, N1], F32, tag="t1a")
                            nc.tensor.transpose(pta[:], g_src[:, bass.ds(j * N2, 128)], ident[0:N1, 0:N1])
                            nc.scalar.copy(out=dst_a[:, bass.ts(c, N1)], in_=pta[:])
                            ptb = pt1.tile([41, N1], F32, tag="t1b")
                            nc.tensor.transpose(ptb[:], g_src[:, bass.ds(j * N2 + 128, 41)], ident[0:N1, 0:N1])
                            nc.scalar.copy(out=dst_b[:, bass.ts(c, N1)], in_=ptb[:])

            # bulk bf16 casts of GtT for the Im-path matmuls (off chain)
            nc.vector.tensor_copy(gtt_reh_a[:], gtt_re_a[:])
            nc.gpsimd.tensor_copy(gtt_reh_b[:], gtt_re_b[:])
            nc.vector.tensor_copy(gtt_imh_a[:], gtt_im_a[:])
            nc.gpsimd.tensor_copy(gtt_imh_b[:], gtt_im_b[:])

            # ---------- FWD stage 2 + mask + INV stage 1 + twiddle2 + T2 ----------
            with tc.tile_pool(name="ps2", bufs=2, space="PSUM") as ps2, \
                 tc.tile_pool(name="pi1", bufs=2, space="PSUM") as pi1, \
                 tc.tile_pool(name="pt2", bufs=2, space="PSUM") as pt2:
                QM = 85
                for f in range(NF2):
                    sl = bass.ts(f, FS2)
                    ps_xre = ps2.tile([QM, FS2], F32, tag="s2re")
                    ps_xim = ps2.tile([QM, FS2], F32, tag="s2im", bufs=1)
                    lc_a = ct["c169c"][0][:, 0:QM]
                    lc_b = ct["c169c"][128][:, 0:QM]
                    ls_a = ct["c169s"][0][:, 0:QM]
                    ls_b = ct["c169s"][128][:, 0:QM]
                    lsn_a = ct["c169sn"][0][:, 0:QM]
                    lsn_b = ct["c169sn"][128][:, 0:QM]
                    # Xre = Cc.T@GtTre + Cs.T@GtTim   (fp32)
                    nc.tensor.matmul(ps_xre[:], lc_a, gtt_re_a[:, sl], start=True, stop=False)
                    nc.tensor.matmul(ps_xre[:], lc_b, gtt_re_b[:, sl], start=False, stop=False)
                    nc.tensor.matmul(ps_xre[:], ls_a, gtt_im_a[:, sl], start=False, stop=False)
                    nc.tensor.matmul(ps_xre[:], ls_b, gtt_im_b[:, sl], start=False, stop=True)
                    # Xim = Cc.T@GtTim - Cs.T@GtTre   (bf16: only shapes delta)
                    nc.tensor.matmul(ps_xim[:], ct["c169ch"][0][:, 0:QM], gtt_imh_a[:, sl], start=True, stop=False)
                    nc.tensor.matmul(ps_xim[:], ct["c169ch"][128][:, 0:QM], gtt_imh_b[:, sl], start=False, stop=False)
                    nc.tensor.matmul(ps_xim[:], ct["c169snh"][0][:, 0:QM], gtt_reh_a[:, sl], start=False, stop=False)
                    nc.tensor.matmul(ps_xim[:], ct["c169snh"][128][:, 0:QM], gtt_reh_b[:, sl], start=False, stop=True)
                    pm = chk.tile([QM, FS2], F32, tag="pm")
                    nc.scalar.activation(pm[:], ps_xre[:],
                                         mybir.ActivationFunctionType.Abs)
                    nc.vector.tensor_scalar(
                        out=pm[:], in0=pm[:], scalar1=THRESH, scalar2=None,
                        op0=mybir.AluOpType.is_lt)
                    nc.vector.tensor_mul(out=pm[:], in0=pm[:], in1=ct["wgtx"][:])
                    zr = chk.tile([QM, FS2], F32R, tag="zre")
                    zi = chk.tile([QM, FS2], F32R, tag="zim")
                    nc.vector.tensor_mul(out=zr[:], in0=ps_xre[:], in1=pm[:])
                    nc.vector.tensor_mul(out=zi[:], in0=ps_xim[:], in1=pm[:])

                    for (b0, bn) in QCH:
                        ps_hre = pi1.tile([128, FS2], F32, tag="i1re")
                        ps_him = pi1.tile([128, FS2], F32, tag="i1im", bufs=1)
                        lc = ct["c169cr"][0][0:QM, bass.ds(b0, bn)]
                        ls = ct["c169sr"][0][0:QM, bass.ds(b0, bn)]
                        lsn = ct["c169snr"][0][0:QM, bass.ds(b0, bn)]
                        # Hre = Cc.T@Zre - Cs.T@Zim ; Him = Cs.T@Zre + Cc.T@Zim
                        nc.tensor.matmul(ps_hre[0:bn], lc, zr[:], start=True, stop=False)
                        nc.tensor.matmul(ps_hre[0:bn], lsn, zi[:], start=False, stop=True)
                        nc.tensor.matmul(ps_him[0:bn], ls, zr[:], start=True, stop=False)
                        nc.tensor.matmul(ps_him[0:bn], lc, zi[:], start=False, stop=True)
                        htw_re = chk.tile([128, FS2], BF16, tag="htw_re")
                        htw_im = chk.tile([128, FS2], BF16, tag="htw_im")
                        t2c_sl = ct["t2c"][b0][0:bn, :]
                        t2s_sl = ct["t2s"][b0][0:bn, :]
                        t2sn_sl = ct["t2sn"][b0][0:bn, :]
                        for j in range(4):
                            cs = bass.ts(j, N1)
                            tmp2 = chk.tile([128, N1], F32, tag="tw2tmp")
                            tmp3 = chk.tile([128, N1], F32, tag="tw2tmp3")
                            tmp4 = chk.tile([128, N1], F32, tag="tw2tmp4")
                            tmp5 = chk.tile([128, N1], F32, tag="tw2tmp5")
                            # Htwre = Hre*t2c + Him*(-t2s) ; Htwim = Hre*t2s + Him*t2c
                            nc.vector.tensor_mul(out=tmp2[0:bn], in0=ps_hre[0:bn, cs], in1=t2c_sl)
                            nc.vector.tensor_mul(out=tmp3[0:bn], in0=ps_him[0:bn, cs], in1=t2sn_sl)
                            nc.gpsimd.tensor_add(out=htw_re[0:bn, cs], in0=tmp2[0:bn], in1=tmp3[0:bn])
                            nc.vector.tensor_mul(out=tmp4[0:bn], in0=ps_hre[0:bn, cs], in1=t2s_sl)
                            nc.vector.tensor_mul(out=tmp5[0:bn], in0=ps_him[0:bn, cs], in1=t2c_sl)
                            nc.gpsimd.tensor_add(out=htw_im[0:bn, cs], in0=tmp4[0:bn], in1=tmp5[0:bn])
                        for j in range(4):
                            c = 4 * f + j
                            for src, dst in ((htw_re, htwt_re), (htw_im, htwt_im)):
                                pt = pt2.tile([N1, 128], BF16, tag="t2p")
                                nc.tensor.transpose(pt[:, 0:bn], src[0:bn, bass.ts(j, N1)], identh[0:bn, 0:bn])
                                nc.scalar.copy(out=dst[:, bass.ds(c * N2 + b0, bn)], in_=pt[:, 0:bn])

            gttp_cm.__exit__(None, None, None)
            xtcp_cm = tc.tile_pool(name="xtcp", bufs=1)
            xtcp = xtcp_cm.__enter__()
            xtq = [xtcp.tile([96, QSPAN], F32R, tag=f"xtq{g}", name=f"xtq{g}")
                   for g in range(4)]
            xtr = xtcp.tile([N1, S * N2], F32R, tag="xtr", name="xtr")

            # ---------- INV stage 2 + subtract (in place into x0t) ----------
            with tc.tile_pool(name="ps3", bufs=2, space="PSUM") as ps3:
                for f in range(NF1):
                    sl = bass.ts(f, FS1)
                    ps_d = ps3.tile([N1, FS1], F32, tag="dlt")
                    nc.tensor.matmul(ps_d[:], ct["cinvc"][:], htwt_re[:, sl], start=True, stop=False)
                    nc.tensor.matmul(ps_d[:], ct["cinvsn"][:], htwt_im[:, sl], start=False, stop=True)
                    nc.vector.tensor_sub(out=xtr[:, sl], in0=x0t[:, sl], in1=ps_d[:])

                # flatten per channel to DRAM [c, 16900], then per quarter
                # gather the three shift groups with one wide DMA each
                for c in range(C):
                    nc.sync.dma_start(out=xdram[c:c + 1, :],
                                      in_=xtr[:, bass.ts(c, N2)])
                for g in range(4):
                    a0, a1 = QA[g]
                    span = (a1 - a0) * N2
                    for s in range(3):
                        nc.sync.dma_start(
                            out=xtq[g][32 * s:32 * s + 32, 0:span - s],
                            in_=xdram[:, N2 * a0 + s:N2 * a0 + span])

                # ---------- conv 3x3 valid on 130x130 + bias ----------
                for p in range(32):
                    g, lp = p // 8, p % 8
                    a0, _ = QA[g]
                    ps_o = ps3.tile([OH, 512], F32, tag="conv")
                    for r in range(3):
                        # image row = 32g + 4*lp + r, flat offset rebased to 169*a0
                        off0 = (32 * g + 4 * lp + r) * W130 - N2 * a0
                        rhs = xtq[g][0:96, off0:off0 + 4 * W130] \
                            .rearrange("c (i w) -> c i w", w=W130)[:, :, 0:128]
                        lhsT = wk_t[:, bass.ts(r, OH)]
                        nc.tensor.matmul(ps_o[:].rearrange("o (i t) -> o i t", t=128),
                                         lhsT, rhs,
                                         start=(r == 0), stop=(r == 2))
                    ost = chk.tile([OH, 512], F32, tag="ost")
                    nc.scalar.activation(ost[:], ps_o[:],
                                         mybir.ActivationFunctionType.Identity,
                                         bias=bias_t[:], scale=1.0)
                    nc.sync.dma_start(out=out_part[:, bass.ts(p, 512)], in_=ost[:])

            xtcp_cm.__exit__(None, None, None)
            htwtp_cm.__exit__(None, None, None)

    _split_excess_waits(nc)
    return nc


_NC_CACHE = {}


def _get_nc():
    if "nc" not in _NC_CACHE:
        _NC_CACHE["nc"] = _build()
    return _NC_CACHE["nc"]


def _consts():
    if "consts" in _NC_CACHE:
        return _NC_CACHE["consts"]
    r = np.arange(N1)
    q = np.arange(N2)
    a100 = 2 * np.pi * np.outer(r, r) / N1
    a169 = 2 * np.pi * np.outer(q, q) / N2
    t1 = 2 * np.pi * np.outer(r, q) / N       # [r, b]
    cc = {
        "c100c": np.cos(a100), "c100sn": -np.sin(a100),
        "c169c": np.cos(a169), "c169s": np.sin(a169), "c169sn": -np.sin(a169),
        "t1c": np.cos(t1), "t1s": np.sin(t1), "t1sn": -np.sin(t1),
        "t2c": np.cos(t1).T.copy(), "t2s": np.sin(t1).T.copy(),   # [b', r]
        "t2sn": -np.sin(t1).T.copy(),
        "cinvc": np.cos(a100) / N, "cinvsn": -np.sin(a100) / N,
    }
    # conjugate-symmetry doubling weights for the half-spectrum q in [0,85):
    # w=2 except DC (0,0) and the self-paired q=84 block (r>0) which get w=1
    wgt = np.full((85, N1), 2.0)
    wgt[0, 0] = 1.0
    wgt[84, 1:] = 1.0
    cc["wgtx"] = np.tile(wgt, (1, 4))
    cc = {k: np.ascontiguousarray(v, dtype=np.float32) for k, v in cc.items()}
    _NC_CACHE["consts"] = cc
    return cc


def kernel(x, weight, bias):
    x = np.asarray(x, dtype=np.float32)
    weight = np.asarray(weight, dtype=np.float32)
    bias = np.asarray(bias, dtype=np.float32)
    nc = _get_nc()
    cc = _consts()

    xp = np.pad(x, ((0, 0), (0, 0), (1, 1), (1, 1)))          # (4,32,130,130)
    # [a, (c, b)] layout of the flat 16900 signal, per batch
    x0s = [np.ascontiguousarray(
        xp[b].reshape(C, N).reshape(C, N1, N2).transpose(1, 0, 2).reshape(N1, C * N2))
        for b in range(B)]

    in_maps = []
    for core in range(8):
        b, h = core // 2, core % 2
        o0 = h * OH
        wkm = np.empty((96, 3 * OH), dtype=np.float32)
        for s in range(3):
            for r in range(3):
                wkm[32 * s:32 * s + 32, r * OH:(r + 1) * OH] = weight[o0:o0 + OH, :, r, s].T
        m = {"x0": x0s[b], "wk": wkm,
             "bias_h": np.ascontiguousarray(bias[o0:o0 + OH, None])}
        m.update(cc)
        in_maps.append(m)

    res = run_bass_kernel_spmd(nc, in_maps, core_ids=list(range(8)))

    out = np.empty((B, O, 128, 128), dtype=np.float32)
    for core in range(8):
        b, h = core // 2, core % 2
        out[b, h * OH:(h + 1) * OH] = res.results[core]["out_part"].reshape(OH, 128, 128)
    return out
